# revision 17
# baseline (speedup 1.0000x reference)
"""Bahdanau multi-head attention on 8 Trainium2 NeuronCores.

Sharding: 8 shards = (batch B=4) x (query-half Lq=128). Each core owns ALL
heads for its 128 query rows, so the W0 output projection is fully local.

Math: tanh(q+k) is expanded in a 6-term sine series at odd harmonics of a
base frequency (tanh(x) ~ sum_m c_m sin((2m-1) w x)), which turns the
Bahdanau score contraction into PE matmuls over (sin/cos feature, dk)
pairs instead of a 16.7M-element ScalarE tanh:

  scores[q,k] = sum_d vp_d tanh(qh_d + kh_d)
             ~= sum_m c_m sum_d vp_d [sin_m(q)cos_m(k) + cos_m(q)sin_m(k)]

Per-core device algorithm (L=256 keys, H=8 heads, DK=64):
  1. Duplicated projections on PE (fp16, FWL): psum qh2[h] = [128=(2 x dk),
     128 q] per head (dk rows duplicated so sin/cos share one tile), ACT
     Copy+per-head-bias moves them to SBUF fp16. Same for kh2 (256 cols).
  2. Seed features via ACT Sin with per-partition phase {0, pi/2}:
     F1 = [sin(w x); cos(w x)] (args <= 3.06 < pi, inside the Sin table's
     valid range). ACT Square + DVE tensor_scalar give C2 = 2cos(2 w x).
  3. Chebyshev ladder on DVE (fp16 2x): F3 = F1*(C2 +- 1),
     F{m+2} = C2*Fm - F{m-2} produces odd harmonics {1,3,5,7,9,11}.
     vp folds into the q-side seed, c_m applied per harmonic.
  4. Scores: 6 matmuls per head accumulate [128 q, 256 k] f32 in PSUM with
     queries on partitions - the natural softmax orientation, no transpose.
  5. Softmax without max-subtraction (|scores| <= ~6 so exp is safe):
     ACT Exp straight from PSUM, DVE rowsum/reciprocal/scale.
  6. attn transposed via plain matmul against identity (faster than
     transpose-mode), attn @ vh on PE, W0 projection, +b0, DMA out.
"""

import numpy as np

B, L, D, H, DK = 4, 256, 512, 8, 64
NCORES = 8
QT = 128          # query rows per core
NCH = D // 128    # 4 chunks of 128 along D
NEG = -30.0       # masked-score penalty (exp(-30) ~ 1e-13)

OM_HALF = 0.300
HARMONICS = (1, 3, 5, 7, 9, 11)
M = len(HARMONICS)


def _fit_coeffs():
    om = np.array([m * OM_HALF for m in HARMONICS])
    x = np.linspace(-9.9, 9.9, 4001)
    wgt = np.exp(-x * x / 4.0) + 3e-3
    A = np.sin(np.outer(x, om))
    w = np.sqrt(wgt)[:, None]
    c, *_ = np.linalg.lstsq(A * w, np.tanh(x) * w[:, 0], rcond=None)
    return c.astype(np.float64)


_C = _fit_coeffs()

_compiled = {}


def _build_nc(masked):
    import concourse.mybir as mybir
    import concourse.tile as tile
    from concourse import bacc

    f32 = mybir.dt.float32
    f16 = mybir.dt.float16
    AF = mybir.ActivationFunctionType
    ALU = mybir.AluOpType
    AX = mybir.AxisListType

    nc = bacc.Bacc(
        "TRN2",
        target_bir_lowering=False,
        debug=False,
        enable_asserts=False,
        num_devices=NCORES,
    )

    qT = nc.dram_tensor("qT", [D, QT], f16, kind="ExternalInput").ap()
    kT = nc.dram_tensor("kT", [D, L], f16, kind="ExternalInput").ap()
    vT = nc.dram_tensor("vT", [D, L], f16, kind="ExternalInput").ap()
    Wqd = nc.dram_tensor("Wqd", [D, H * 128], f16, kind="ExternalInput").ap()
    Wkd = nc.dram_tensor("Wkd", [D, H * 128], f16, kind="ExternalInput").ap()
    Wv = nc.dram_tensor("Wv", [D, D], f16, kind="ExternalInput").ap()
    W0 = nc.dram_tensor("W0", [D, D], f16, kind="ExternalInput").ap()
    # per-(partition, head) seed bias: OM_HALF*b(q|k)_dup + phase
    sbq = nc.dram_tensor("sbq", [128, H], f32, kind="ExternalInput").ap()
    sbk = nc.dram_tensor("sbk", [128, H], f32, kind="ExternalInput").ap()
    bvB = nc.dram_tensor("bvB", [128, D], f32, kind="ExternalInput").ap()
    b0B = nc.dram_tensor("b0B", [128, D], f32, kind="ExternalInput").ap()
    vpdup = nc.dram_tensor("vpdup", [128, H], f32, kind="ExternalInput").ap()
    # per-partition ladder constants, cols 0-2 q side / 3-5 k side:
    # [mp, ap, pm1] with halves swapped between sides (q = [sin; cos],
    # k = [cos; sin])
    ppc = nc.dram_tensor("ppc", [128, 6], f32, kind="ExternalInput").ap()
    ident = nc.dram_tensor("ident", [128, 128], f16, kind="ExternalInput").ap()
    if masked:
        pen = nc.dram_tensor("pen", [QT, L], f32, kind="ExternalInput").ap()
    out = nc.dram_tensor("out", [QT, D], f32, kind="ExternalOutput").ap()

    NQ = H * QT   # 1024 q-side feature cols
    NK = H * L    # 2048 k-side feature cols

    with tile.TileContext(nc) as tc:
        with tc.tile_pool(name="const", bufs=1) as cp:
            qT_t = [cp.tile([128, QT], f16, tag=f"qT{c}", name=f"qT{c}") for c in range(NCH)]
            kT_t = [cp.tile([128, L], f16, tag=f"kT{c}", name=f"kT{c}") for c in range(NCH)]
            vT_t = [cp.tile([128, L], f16, tag=f"vT{c}", name=f"vT{c}") for c in range(NCH)]
            Wqd_t = [cp.tile([128, NQ], f16, tag=f"Wqd{c}", name=f"Wqd{c}") for c in range(NCH)]
            Wkd_t = [cp.tile([128, NQ], f16, tag=f"Wkd{c}", name=f"Wkd{c}") for c in range(NCH)]
            Wv_t = [cp.tile([128, D], f16, tag=f"Wv{c}", name=f"Wv{c}") for c in range(NCH)]
            W0_t = [cp.tile([128, D], f16, tag=f"W0{c}", name=f"W0{c}") for c in range(NCH)]
            for c in range(NCH):
                cs = slice(c * 128, (c + 1) * 128)
                nc.sync.dma_start(qT_t[c][:], qT[cs, :])
                nc.sync.dma_start(kT_t[c][:], kT[cs, :])
                nc.sync.dma_start(vT_t[c][:], vT[cs, :])
                nc.sync.dma_start(Wqd_t[c][:], Wqd[cs, :])
                nc.sync.dma_start(Wkd_t[c][:], Wkd[cs, :])
                nc.sync.dma_start(Wv_t[c][:], Wv[cs, :])
                nc.sync.dma_start(W0_t[c][:], W0[cs, :])
            sbq_t = cp.tile([128, H], f32, tag="sbq")
            nc.sync.dma_start(sbq_t[:], sbq[:])
            sbk_t = cp.tile([128, H], f32, tag="sbk")
            nc.sync.dma_start(sbk_t[:], sbk[:])
            bvB_t = cp.tile([128, D], f32, tag="bvB")
            nc.sync.dma_start(bvB_t[:], bvB[:])
            b0B_t = cp.tile([128, D], f32, tag="b0B")
            nc.sync.dma_start(b0B_t[:], b0B[:])
            vpdup_t = cp.tile([128, H], f32, tag="vpdup")
            nc.sync.dma_start(vpdup_t[:], vpdup[:])
            ppc_t = cp.tile([128, 6], f32, tag="ppc")
            nc.sync.dma_start(ppc_t[:], ppc[:])
            ident_t = cp.tile([128, 128], f16, tag="ident")
            nc.sync.dma_start(ident_t[:], ident[:])
            if masked:
                pen_t = cp.tile([QT, L], f32, tag="pen")
                nc.sync.dma_start(pen_t[:], pen[:])

            vh16 = [cp.tile([128, D], f16, tag=f"vh{j}", name=f"vh{j}") for j in range(2)]
            F1q = cp.tile([128, NQ], f16, tag="F1q")
            F1k = cp.tile([128, NK], f16, tag="F1k")

            # ---- duplicated projections fused with seed Sin ----
            with (
                tc.tile_pool(name="psq", bufs=2, space="PSUM") as psqp,
                tc.tile_pool(name="psk", bufs=2, space="PSUM") as pskp,
                tc.tile_pool(name="psv", bufs=2, space="PSUM") as psvp,
            ):
                for h in range(H):
                    hs = slice(h * 128, (h + 1) * 128)
                    psq = psqp.tile([128, QT], f32, tag="psq", name=f"psq{h}")
                    for c in range(NCH):
                        nc.tensor.matmul(
                            psq[:], lhsT=Wqd_t[c][:, hs], rhs=qT_t[c][:],
                            start=(c == 0), stop=(c == NCH - 1),
                        )
                    nc.scalar.activation(
                        F1q[:, hs], psq[:], AF.Sin,
                        bias=sbq_t[:, h : h + 1], scale=OM_HALF,
                    )
                    ks = slice(h * L, (h + 1) * L)
                    psk = pskp.tile([128, L], f32, tag="psk", name=f"psk{h}")
                    for c in range(NCH):
                        nc.tensor.matmul(
                            psk[:], lhsT=Wkd_t[c][:, hs], rhs=kT_t[c][:],
                            start=(c == 0), stop=(c == NCH - 1),
                        )
                    nc.scalar.activation(
                        F1k[:, ks], psk[:], AF.Sin,
                        bias=sbk_t[:, h : h + 1], scale=OM_HALF,
                    )
                for j in range(2):
                    js = slice(j * 128, (j + 1) * 128)
                    psv = psvp.tile([128, D], f32, tag="psv", name=f"psv{j}")
                    for c in range(NCH):
                        nc.tensor.matmul(
                            psv[:], lhsT=vT_t[c][:, js], rhs=Wv_t[c][:],
                            start=(c == 0), stop=(c == NCH - 1),
                        )
                    nc.vector.tensor_add(vh16[j][:], psv[:], bvB_t[:])

            # ---- ladder multipliers: C2 = 2cos(2 w x), M3 = C2 -+ 1 ----
            def mults(F1, n, sfx, o):
                mp = ppc_t[:, o : o + 1]
                ap = ppc_t[:, o + 1 : o + 2]
                pm1 = ppc_t[:, o + 2 : o + 3]
                SQ = cp.tile([128, n], f16, tag=f"SQ{sfx}", name=f"SQ{sfx}")
                nc.vector.tensor_mul(SQ[:], F1[:], F1[:])
                C2 = cp.tile([128, n], f16, tag=f"C2{sfx}", name=f"C2{sfx}")
                nc.vector.tensor_scalar(C2[:], SQ[:], mp, ap, ALU.mult, ALU.add)
                M3 = cp.tile([128, n], f16, tag=f"M3{sfx}", name=f"M3{sfx}")
                nc.vector.tensor_scalar(M3[:], C2[:], pm1, None, ALU.add)
                return C2, M3

            C2q, M3q = mults(F1q, NQ, "q", 0)
            C2k, M3k = mults(F1k, NK, "k", 3)

            Gq = [cp.tile([128, NQ], f16, tag=f"Gq{i}", name=f"Gq{i}") for i in range(M)]
            Hq = [cp.tile([128, NQ], f16, tag=f"Hq{i}", name=f"Hq{i}") for i in range(M)]
            Kk = [None] * M
            Kk[0] = F1k
            Kk[1] = cp.tile([128, NK], f16, tag="Kk1", name="Kk1")
            for i in range(2, M):
                Kk[i] = cp.tile([128, NK], f16, tag=f"Kk{i}", name=f"Kk{i}")
            tmpq = [cp.tile([128, NQ], f16, tag=f"tq{i}", name=f"tq{i}") for i in range(M - 2)]
            tmpk = [cp.tile([128, NK], f16, tag=f"tk{i}", name=f"tk{i}") for i in range(M - 2)]

            # ---- ladder + scores, interleaved per harmonic so the PE
            # chases the DVE (both engines are strict FIFO) ----
            with (
                tc.tile_pool(name="sc", bufs=4, space="PSUM") as scp,
                tc.tile_pool(name="tr", bufs=2, space="PSUM") as trp,
                tc.tile_pool(name="av", bufs=2, space="PSUM") as avp,
                tc.tile_pool(name="smp", bufs=2) as smp,
            ):
                # two heads per PSUM bank: head h lives in sc_ps[h//2]
                # cols [(h%2)*L, (h%2+1)*L). start=True clears has_written
                # bank-wide, so only the bank's FIRST matmul (h even, i=0)
                # sets it; the odd head's first write lands on cleared bits
                # and overwrites correctly with start=False.
                sc_ps = [
                    scp.tile([128, 2 * L], f32, tag="sc", name=f"sc{hp}")
                    for hp in range(H // 2)
                ]

                def sc_ap(h):
                    return sc_ps[h // 2][:, (h % 2) * L : (h % 2 + 1) * L]

                def score_mms(i):
                    for h in range(H):
                        nc.tensor.matmul(
                            sc_ap(h),
                            lhsT=Hq[i][:, h * 128 : (h + 1) * 128],
                            rhs=Kk[i][:, h * L : (h + 1) * L],
                            start=(i == 0 and h % 2 == 0),
                            stop=(i == M - 1),
                        )

                for i in range(M):
                    if i == 0:
                        for h in range(H):
                            hs = slice(h * 128, (h + 1) * 128)
                            nc.vector.tensor_scalar_mul(
                                Gq[0][:, hs], F1q[:, hs], vpdup_t[:, h : h + 1]
                            )
                    elif i == 1:
                        nc.vector.tensor_mul(Gq[1][:], Gq[0][:], M3q[:])
                        nc.vector.tensor_mul(Kk[1][:], F1k[:], M3k[:])
                    else:
                        nc.vector.tensor_mul(tmpq[i - 2][:], C2q[:], Gq[i - 1][:])
                        nc.vector.tensor_sub(Gq[i][:], tmpq[i - 2][:], Gq[i - 2][:])
                        nc.vector.tensor_mul(tmpk[i - 2][:], C2k[:], Kk[i - 1][:])
                        nc.vector.tensor_sub(Kk[i][:], tmpk[i - 2][:], Kk[i - 2][:])
                    nc.vector.tensor_scalar_mul(Hq[i][:], Gq[i][:], float(_C[i]))
                    score_mms(i)

                # cos(q)sin(k) pairing: Hq holds [c_m vp sin; c_m vp cos] and
                # Kk holds [cos; sin] after the phase layout below, so one
                # matmul per (m, h) covers both product terms.

                aoT = [cp.tile([128, QT], f16, tag=f"aoT{c}", name=f"aoT{c}") for c in range(NCH)]
                for h in range(H):
                    if masked:
                        spen = smp.tile([QT, L], f32, tag="spen")
                        nc.vector.tensor_add(spen[:], sc_ap(h), pen_t[:])
                        p = smp.tile([QT, L], f16, tag="p")
                        nc.scalar.activation(p[:], spen[:], AF.Exp)
                    else:
                        p = smp.tile([QT, L], f16, tag="p")
                        nc.scalar.activation(p[:], sc_ap(h), AF.Exp)
                    rs = smp.tile([QT, 1], f32, tag="rs")
                    nc.vector.tensor_reduce(rs[:], p[:], axis=AX.X, op=ALU.add)
                    rcp = smp.tile([QT, 1], f32, tag="rcp")
                    nc.vector.reciprocal(rcp[:], rs[:])
                    attn = smp.tile([QT, L], f16, tag="attn")
                    nc.vector.tensor_scalar_mul(attn[:], p[:], rcp[:])
                    av = avp.tile([64, QT], f32, tag="av", name=f"av{h}")
                    for j in range(2):
                        js = slice(j * 128, (j + 1) * 128)
                        tr = trp.tile([128, QT], f32, tag="tr")
                        nc.tensor.matmul(
                            tr[:], lhsT=attn[:, js], rhs=ident_t[:],
                            start=True, stop=True,
                        )
                        attnT = smp.tile([128, QT], f16, tag="attnT")
                        nc.vector.tensor_copy(attnT[:], tr[:])
                        nc.tensor.matmul(
                            av[:], lhsT=vh16[j][:, h * 64 : (h + 1) * 64],
                            rhs=attnT[:], start=(j == 0), stop=(j == 1),
                        )
                    hh = h % 2
                    nc.vector.tensor_copy(
                        aoT[h // 2][hh * 64 : (hh + 1) * 64, :], av[:]
                    )

            # ---- output projection ----
            with (
                tc.tile_pool(name="ops", bufs=1, space="PSUM") as opsp,
                tc.tile_pool(name="outp", bufs=1) as outp,
            ):
                ops = opsp.tile([QT, D], f32, tag="ops")
                for c in range(NCH):
                    nc.tensor.matmul(
                        ops[:], lhsT=aoT[c][:], rhs=W0_t[c][:],
                        start=(c == 0), stop=(c == NCH - 1),
                    )
                y = outp.tile([QT, D], f32, tag="y")
                nc.vector.tensor_add(y[:], ops[:], b0B_t[:])
                nc.sync.dma_start(out[:], y[:])

    nc.compile()
    return nc


def _dup_cols(W):
    # [D, D] -> [D, H*128] with cols h*128 + r*64 + j = W[:, h*64 + j]
    idx = np.arange(H * 128)
    src = (idx // 128) * 64 + (idx % 64)
    return np.ascontiguousarray(W[:, src])


def _dup_part(v):
    # [D] -> [128, H] with [p, h] = v[h*64 + p%64]
    p = np.arange(128) % 64
    return np.ascontiguousarray(v[p[:, None] + np.arange(H)[None, :] * 64])


def build_in_maps(q, k, v, mask, Wq, bq, Wk, bk, Wv, bv, vp, W0, b0):
    q = np.asarray(q, np.float32)
    k = np.asarray(k, np.float32)
    v = np.asarray(v, np.float32)
    mask = np.asarray(mask)
    vp = np.asarray(vp, np.float32).reshape(H, DK)

    masked = not bool(np.all(mask != 0))

    ppc = np.zeros((128, 6), np.float32)
    # q side ([sin; cos]): mp {-4, 4}, ap {2, -2}, pm1 {1, -1}
    ppc[:64, 0], ppc[64:, 0] = -4.0, 4.0
    ppc[:64, 1], ppc[64:, 1] = 2.0, -2.0
    ppc[:64, 2], ppc[64:, 2] = 1.0, -1.0
    # k side ([cos; sin]): halves swapped
    ppc[:64, 3], ppc[64:, 3] = 4.0, -4.0
    ppc[:64, 4], ppc[64:, 4] = -2.0, 2.0
    ppc[:64, 5], ppc[64:, 5] = -1.0, 1.0

    phq = np.zeros((128, 1), np.float32)
    phq[64:] = np.pi / 2                 # q: [sin; cos]
    phk = np.zeros((128, 1), np.float32)
    phk[:64] = np.pi / 2                 # k: [cos; sin]

    shared = dict(
        Wqd=_dup_cols(np.asarray(Wq, np.float32)).astype(np.float16),
        Wkd=_dup_cols(np.asarray(Wk, np.float32)).astype(np.float16),
        Wv=np.ascontiguousarray(np.asarray(Wv, np.float32)).astype(np.float16),
        W0=np.ascontiguousarray(np.asarray(W0, np.float32)).astype(np.float16),
        sbq=OM_HALF * _dup_part(np.asarray(bq, np.float32)) + phq,
        sbk=OM_HALF * _dup_part(np.asarray(bk, np.float32)) + phk,
        bvB=np.ascontiguousarray(
            np.broadcast_to(np.asarray(bv, np.float32), (128, D))
        ),
        b0B=np.ascontiguousarray(
            np.broadcast_to(np.asarray(b0, np.float32), (128, D))
        ),
        vpdup=np.ascontiguousarray(
            vp.T[np.arange(128) % 64][:, :]
        ),
        ppc=ppc,
        ident=np.eye(128, dtype=np.float16),
    )
    in_maps = []
    for c in range(NCORES):
        b, half = c // 2, c % 2
        rows = slice(half * QT, (half + 1) * QT)
        m = dict(shared)
        m["qT"] = np.ascontiguousarray(q[b, rows, :].T).astype(np.float16)
        m["kT"] = np.ascontiguousarray(k[b].T).astype(np.float16)
        m["vT"] = np.ascontiguousarray(v[b].T).astype(np.float16)
        if masked:
            m["pen"] = np.ascontiguousarray(
                np.where(mask[b, rows, :] == 0, NEG, 0.0).astype(np.float32)
            )
        in_maps.append(m)
    return in_maps, masked


def kernel(q, k, v, mask, Wq, bq, Wk, bk, Wv, bv, vp, W0, b0):
    in_maps, masked = build_in_maps(
        q, k, v, mask, Wq, bq, Wk, bk, Wv, bv, vp, W0, b0
    )
    if masked not in _compiled:
        _compiled[masked] = _build_nc(masked)
    from concourse.bass_utils import run_bass_kernel_spmd

    res = run_bass_kernel_spmd(
        _compiled[masked], in_maps, core_ids=list(range(NCORES))
    )
    outf = np.zeros((B, L, D), np.float32)
    for c, r in enumerate(res.results):
        b, half = c // 2, c % 2
        outf[b, half * QT : (half + 1) * QT, :] = r["out"]
    return outf


# revision 20
# speedup vs baseline: 1.3311x; 1.3311x over previous
"""Bahdanau multi-head attention on 8 Trainium2 NeuronCores.

Sharding: 8 shards = (batch B=4) x (query-half Lq=128). Each core owns ALL
heads for its 128 query rows, so the W0 output projection is fully local.

Math: tanh(q+k) is expanded in a 6-term sine series at odd harmonics of a
base frequency (tanh(x) ~ sum_m c_m sin((2m-1) w x)), which turns the
Bahdanau score contraction into PE matmuls over (sin/cos feature, dk)
pairs instead of a 16.7M-element ScalarE tanh:

  scores[q,k] = sum_d vp_d tanh(qh_d + kh_d)
             ~= sum_m c_m sum_d vp_d [sin_m(q)cos_m(k) + cos_m(q)sin_m(k)]

Per-core device algorithm (L=256 keys, H=8 heads, DK=64):
  1. Duplicated projections on PE (fp16, FWL): psum qh2[h] = [128=(2 x dk),
     128 q] per head (dk rows duplicated so sin/cos share one tile), ACT
     Copy+per-head-bias moves them to SBUF fp16. Same for kh2 (256 cols).
  2. Seed features via ACT Sin with per-partition phase {0, pi/2}:
     F1 = [sin(w x); cos(w x)] (args <= 3.06 < pi, inside the Sin table's
     valid range). ACT Square + DVE tensor_scalar give C2 = 2cos(2 w x).
  3. Chebyshev ladder on DVE (fp16 2x): F3 = F1*(C2 +- 1),
     F{m+2} = C2*Fm - F{m-2} produces odd harmonics {1,3,5,7,9,11}.
     vp folds into the q-side seed, c_m applied per harmonic.
  4. Scores: 6 matmuls per head accumulate [128 q, 256 k] f32 in PSUM with
     queries on partitions - the natural softmax orientation, no transpose.
  5. Softmax without max-subtraction (|scores| <= ~6 so exp is safe):
     ACT Exp straight from PSUM, DVE rowsum/reciprocal/scale.
  6. attn transposed via plain matmul against identity (faster than
     transpose-mode), attn @ vh on PE, W0 projection, +b0, DMA out.
"""

import numpy as np

B, L, D, H, DK = 4, 256, 512, 8, 64
NCORES = 8
QT = 128          # query rows per core
NCH = D // 128    # 4 chunks of 128 along D
NEG = -30.0       # masked-score penalty (exp(-30) ~ 1e-13)

OM_HALF = 0.300
HARMONICS = (1, 3, 5, 7, 9, 11)
M = len(HARMONICS)


def _fit_coeffs():
    om = np.array([m * OM_HALF for m in HARMONICS])
    x = np.linspace(-9.9, 9.9, 4001)
    wgt = np.exp(-x * x / 4.0) + 3e-3
    A = np.sin(np.outer(x, om))
    w = np.sqrt(wgt)[:, None]
    c, *_ = np.linalg.lstsq(A * w, np.tanh(x) * w[:, 0], rcond=None)
    return c.astype(np.float64)


_C = _fit_coeffs()

_compiled = {}


def _build_nc(masked):
    import concourse.mybir as mybir
    import concourse.tile as tile
    from concourse import bacc

    f32 = mybir.dt.float32
    f16 = mybir.dt.float16
    AF = mybir.ActivationFunctionType
    ALU = mybir.AluOpType
    AX = mybir.AxisListType

    nc = bacc.Bacc(
        "TRN2",
        target_bir_lowering=False,
        debug=False,
        enable_asserts=False,
        num_devices=NCORES,
    )

    # consolidated input blobs - one DMA each (HWDGE charges ~625ns per
    # DMA regardless of size, so 35 small DMAs would serialize ~22us):
    # blob32: [sbq 8 | sbk 8 | vpdup 8 | ppc 6 | bvB 512 | b0B 512]
    # blobA:  [Wqd 4x1024 | qT 4x128 | ident 128]
    # blobB:  [Wkd 4x1024 | kT 4x256]
    # blobC:  [Wv 4x512 | vT 4x256 | W0 4x512]
    N32 = 1054
    NA = 4 * 1024 + 4 * QT + 128
    NB = 4 * 1024 + 4 * L
    NC = 2048 + 4 * L + 2048
    blob32 = nc.dram_tensor("blob32", [128, N32], f32, kind="ExternalInput").ap()
    blobA = nc.dram_tensor("blobA", [128, NA], f16, kind="ExternalInput").ap()
    blobB = nc.dram_tensor("blobB", [128, NB], f16, kind="ExternalInput").ap()
    blobC = nc.dram_tensor("blobC", [128, NC], f16, kind="ExternalInput").ap()
    if masked:
        pen = nc.dram_tensor("pen", [QT, L], f32, kind="ExternalInput").ap()
    out = nc.dram_tensor("out", [QT, D], f32, kind="ExternalOutput").ap()

    NQ = H * QT   # 1024 q-side feature cols
    NK = H * L    # 2048 k-side feature cols

    with tile.TileContext(nc) as tc:
        with tc.tile_pool(name="const", bufs=1) as cp:
            t32 = cp.tile([128, N32], f32, tag="t32")
            nc.sync.dma_start(t32[:], blob32[:])
            tA = cp.tile([128, NA], f16, tag="tA")
            nc.sync.dma_start(tA[:], blobA[:])
            tB = cp.tile([128, NB], f16, tag="tB")
            nc.sync.dma_start(tB[:], blobB[:])
            tC = cp.tile([128, NC], f16, tag="tC")
            nc.sync.dma_start(tC[:], blobC[:])
            if masked:
                pen_t = cp.tile([QT, L], f32, tag="pen")
                nc.sync.dma_start(pen_t[:], pen[:])

            sbq_t = t32[:, 0:H]
            sbk_t = t32[:, H : 2 * H]
            vpdup_t = t32[:, 2 * H : 3 * H]
            ppc_t = t32[:, 3 * H : 3 * H + 6]
            bvB_t = t32[:, 30 : 30 + D]
            b0B_t = t32[:, 30 + D : 30 + 2 * D]
            Wqd_t = [tA[:, c * 1024 : (c + 1) * 1024] for c in range(NCH)]
            qT_t = [tA[:, 4096 + c * QT : 4096 + (c + 1) * QT] for c in range(NCH)]
            ident_t = tA[:, 4096 + 4 * QT : 4096 + 4 * QT + 128]
            Wkd_t = [tB[:, c * 1024 : (c + 1) * 1024] for c in range(NCH)]
            kT_t = [tB[:, 4096 + c * L : 4096 + (c + 1) * L] for c in range(NCH)]
            Wv_t = [tC[:, c * 512 : (c + 1) * 512] for c in range(NCH)]
            vT_t = [tC[:, 2048 + c * L : 2048 + (c + 1) * L] for c in range(NCH)]
            W0_t = [tC[:, 3072 + c * 512 : 3072 + (c + 1) * 512] for c in range(NCH)]

            vh16 = [cp.tile([128, D], f16, tag=f"vh{j}", name=f"vh{j}") for j in range(2)]
            F1q = cp.tile([128, NQ], f16, tag="F1q")
            F1k = cp.tile([128, NK], f16, tag="F1k")

            # ---- duplicated projections fused with seed Sin ----
            with (
                tc.tile_pool(name="psq", bufs=2, space="PSUM") as psqp,
                tc.tile_pool(name="psk", bufs=2, space="PSUM") as pskp,
                tc.tile_pool(name="psv", bufs=2, space="PSUM") as psvp,
            ):
                for h in range(H):
                    hs = slice(h * 128, (h + 1) * 128)
                    psq = psqp.tile([128, QT], f32, tag="psq", name=f"psq{h}")
                    for c in range(NCH):
                        nc.tensor.matmul(
                            psq[:], lhsT=Wqd_t[c][:, hs], rhs=qT_t[c][:],
                            start=(c == 0), stop=(c == NCH - 1),
                        )
                    nc.scalar.activation(
                        F1q[:, hs], psq[:], AF.Sin,
                        bias=sbq_t[:, h : h + 1], scale=OM_HALF,
                    )
                    ks = slice(h * L, (h + 1) * L)
                    psk = pskp.tile([128, L], f32, tag="psk", name=f"psk{h}")
                    for c in range(NCH):
                        nc.tensor.matmul(
                            psk[:], lhsT=Wkd_t[c][:, hs], rhs=kT_t[c][:],
                            start=(c == 0), stop=(c == NCH - 1),
                        )
                    nc.scalar.activation(
                        F1k[:, ks], psk[:], AF.Sin,
                        bias=sbk_t[:, h : h + 1], scale=OM_HALF,
                    )
                for j in range(2):
                    js = slice(j * 128, (j + 1) * 128)
                    psv = psvp.tile([128, D], f32, tag="psv", name=f"psv{j}")
                    for c in range(NCH):
                        nc.tensor.matmul(
                            psv[:], lhsT=vT_t[c][:, js], rhs=Wv_t[c][:],
                            start=(c == 0), stop=(c == NCH - 1),
                        )
                    nc.vector.tensor_add(vh16[j][:], psv[:], bvB_t[:])

            # ---- ladder multipliers: C2 = 2cos(2 w x), M3 = C2 -+ 1 ----
            def mults(F1, n, sfx, o):
                mp = ppc_t[:, o : o + 1]
                ap = ppc_t[:, o + 1 : o + 2]
                pm1 = ppc_t[:, o + 2 : o + 3]
                SQ = cp.tile([128, n], f16, tag=f"SQ{sfx}", name=f"SQ{sfx}")
                nc.scalar.activation(SQ[:], F1[:], AF.Square)
                C2 = cp.tile([128, n], f16, tag=f"C2{sfx}", name=f"C2{sfx}")
                nc.vector.tensor_scalar(C2[:], SQ[:], mp, ap, ALU.mult, ALU.add)
                M3 = cp.tile([128, n], f16, tag=f"M3{sfx}", name=f"M3{sfx}")
                nc.vector.tensor_scalar(M3[:], C2[:], pm1, None, ALU.add)
                return C2, M3

            C2q, M3q = mults(F1q, NQ, "q", 0)
            C2k, M3k = mults(F1k, NK, "k", 3)

            Gq = [cp.tile([128, NQ], f16, tag=f"Gq{i}", name=f"Gq{i}") for i in range(M)]
            Hq = [cp.tile([128, NQ], f16, tag=f"Hq{i}", name=f"Hq{i}") for i in range(M)]
            Kk = [None] * M
            Kk[0] = F1k
            Kk[1] = cp.tile([128, NK], f16, tag="Kk1", name="Kk1")
            for i in range(2, M):
                Kk[i] = cp.tile([128, NK], f16, tag=f"Kk{i}", name=f"Kk{i}")
            tmpq = [cp.tile([128, NQ], f16, tag=f"tq{i}", name=f"tq{i}") for i in range(M - 2)]
            tmpk = [cp.tile([128, NK], f16, tag=f"tk{i}", name=f"tk{i}") for i in range(M - 2)]

            # ---- ladder + scores, interleaved per harmonic so the PE
            # chases the DVE (both engines are strict FIFO) ----
            with (
                tc.tile_pool(name="sc", bufs=4, space="PSUM") as scp,
                tc.tile_pool(name="tr", bufs=2, space="PSUM") as trp,
                tc.tile_pool(name="av", bufs=2, space="PSUM") as avp,
                tc.tile_pool(name="smp", bufs=2) as smp,
            ):
                # two heads per PSUM bank: head h lives in sc_ps[h//2]
                # cols [(h%2)*L, (h%2+1)*L). start=True clears has_written
                # bank-wide, so only the bank's FIRST matmul (h even, i=0)
                # sets it; the odd head's first write lands on cleared bits
                # and overwrites correctly with start=False.
                sc_ps = [
                    scp.tile([128, 2 * L], f32, tag="sc", name=f"sc{hp}")
                    for hp in range(H // 2)
                ]

                def sc_ap(h):
                    return sc_ps[h // 2][:, (h % 2) * L : (h % 2 + 1) * L]

                def score_mms(i):
                    for h in range(H):
                        nc.tensor.matmul(
                            sc_ap(h),
                            lhsT=Hq[i][:, h * 128 : (h + 1) * 128],
                            rhs=Kk[i][:, h * L : (h + 1) * L],
                            start=(i == 0 and h % 2 == 0),
                            stop=(i == M - 1),
                        )

                for i in range(M):
                    if i == 0:
                        for h in range(H):
                            hs = slice(h * 128, (h + 1) * 128)
                            nc.vector.tensor_scalar_mul(
                                Gq[0][:, hs], F1q[:, hs], vpdup_t[:, h : h + 1]
                            )
                    elif i == 1:
                        nc.vector.tensor_mul(Gq[1][:], Gq[0][:], M3q[:])
                        nc.vector.tensor_mul(Kk[1][:], F1k[:], M3k[:])
                    else:
                        nc.vector.tensor_mul(tmpq[i - 2][:], C2q[:], Gq[i - 1][:])
                        nc.vector.tensor_sub(Gq[i][:], tmpq[i - 2][:], Gq[i - 2][:])
                        nc.vector.tensor_mul(tmpk[i - 2][:], C2k[:], Kk[i - 1][:])
                        nc.vector.tensor_sub(Kk[i][:], tmpk[i - 2][:], Kk[i - 2][:])
                    nc.scalar.activation(Hq[i][:], Gq[i][:], AF.Copy,
                                         scale=float(_C[i]))
                    score_mms(i)

                # cos(q)sin(k) pairing: Hq holds [c_m vp sin; c_m vp cos] and
                # Kk holds [cos; sin] after the phase layout below, so one
                # matmul per (m, h) covers both product terms.

                aoT = [cp.tile([128, QT], f16, tag=f"aoT{c}", name=f"aoT{c}") for c in range(NCH)]
                for h in range(H):
                    if masked:
                        spen = smp.tile([QT, L], f32, tag="spen")
                        nc.vector.tensor_add(spen[:], sc_ap(h), pen_t[:])
                        p = smp.tile([QT, L], f16, tag="p")
                        nc.scalar.activation(p[:], spen[:], AF.Exp)
                    else:
                        p = smp.tile([QT, L], f16, tag="p")
                        nc.scalar.activation(p[:], sc_ap(h), AF.Exp)
                    rs = smp.tile([QT, 1], f32, tag="rs")
                    nc.vector.tensor_reduce(rs[:], p[:], axis=AX.X, op=ALU.add)
                    rcp = smp.tile([QT, 1], f32, tag="rcp")
                    nc.vector.reciprocal_approx_fast(rcp[:], rs[:])
                    attn = smp.tile([QT, L], f16, tag="attn")
                    nc.vector.tensor_scalar_mul(attn[:], p[:], rcp[:])
                    av = avp.tile([64, QT], f32, tag="av", name=f"av{h}")
                    for j in range(2):
                        js = slice(j * 128, (j + 1) * 128)
                        tr = trp.tile([128, QT], f32, tag="tr")
                        nc.tensor.matmul(
                            tr[:], lhsT=attn[:, js], rhs=ident_t[:],
                            start=True, stop=True,
                        )
                        attnT = smp.tile([128, QT], f16, tag="attnT")
                        nc.scalar.activation(attnT[:], tr[:], AF.Copy)
                        nc.tensor.matmul(
                            av[:], lhsT=vh16[j][:, h * 64 : (h + 1) * 64],
                            rhs=attnT[:], start=(j == 0), stop=(j == 1),
                        )
                    hh = h % 2
                    nc.vector.tensor_copy(
                        aoT[h // 2][hh * 64 : (hh + 1) * 64, :], av[:]
                    )

            # ---- output projection ----
            with (
                tc.tile_pool(name="ops", bufs=1, space="PSUM") as opsp,
                tc.tile_pool(name="outp", bufs=1) as outp,
            ):
                ops = opsp.tile([QT, D], f32, tag="ops")
                for c in range(NCH):
                    nc.tensor.matmul(
                        ops[:], lhsT=aoT[c][:], rhs=W0_t[c][:],
                        start=(c == 0), stop=(c == NCH - 1),
                    )
                y = outp.tile([QT, D], f32, tag="y")
                nc.vector.tensor_add(y[:], ops[:], b0B_t[:])
                nc.sync.dma_start(out[:], y[:])

    nc.compile()
    return nc


def _dup_cols(W):
    # [D, D] -> [D, H*128] with cols h*128 + r*64 + j = W[:, h*64 + j]
    idx = np.arange(H * 128)
    src = (idx // 128) * 64 + (idx % 64)
    return np.ascontiguousarray(W[:, src])


def _dup_part(v):
    # [D] -> [128, H] with [p, h] = v[h*64 + p%64]
    p = np.arange(128) % 64
    return np.ascontiguousarray(v[p[:, None] + np.arange(H)[None, :] * 64])


def build_in_maps(q, k, v, mask, Wq, bq, Wk, bk, Wv, bv, vp, W0, b0):
    q = np.asarray(q, np.float32)
    k = np.asarray(k, np.float32)
    v = np.asarray(v, np.float32)
    mask = np.asarray(mask)
    vp = np.asarray(vp, np.float32).reshape(H, DK)

    masked = not bool(np.all(mask != 0))

    # blob32: [sbq 8 | sbk 8 | vpdup 8 | ppc 6 | bvB 512 | b0B 512]
    blob32 = np.zeros((128, 1054), np.float32)
    phq = np.zeros((128, 1), np.float32)
    phq[64:] = np.pi / 2                 # q: [sin; cos]
    phk = np.zeros((128, 1), np.float32)
    phk[:64] = np.pi / 2                 # k: [cos; sin]
    blob32[:, 0:H] = OM_HALF * _dup_part(np.asarray(bq, np.float32)) + phq
    blob32[:, H : 2 * H] = OM_HALF * _dup_part(np.asarray(bk, np.float32)) + phk
    blob32[:, 2 * H : 3 * H] = vp.T[np.arange(128) % 64]
    ppc = blob32[:, 3 * H : 3 * H + 6]
    # q side ([sin; cos]): mp {-4, 4}, ap {2, -2}, pm1 {1, -1}
    ppc[:64, 0], ppc[64:, 0] = -4.0, 4.0
    ppc[:64, 1], ppc[64:, 1] = 2.0, -2.0
    ppc[:64, 2], ppc[64:, 2] = 1.0, -1.0
    # k side ([cos; sin]): halves swapped
    ppc[:64, 3], ppc[64:, 3] = 4.0, -4.0
    ppc[:64, 4], ppc[64:, 4] = -2.0, 2.0
    ppc[:64, 5], ppc[64:, 5] = -1.0, 1.0
    blob32[:, 30:542] = np.asarray(bv, np.float32)[None, :]
    blob32[:, 542:1054] = np.asarray(b0, np.float32)[None, :]

    def chunks(W, n):
        # [512, n] -> [128, 4*n] chunk-major
        return W.reshape(NCH, 128, n).transpose(1, 0, 2).reshape(128, NCH * n)

    Wqd = _dup_cols(np.asarray(Wq, np.float32)).astype(np.float16)
    Wkd = _dup_cols(np.asarray(Wk, np.float32)).astype(np.float16)
    Wv16 = np.asarray(Wv, np.float32).astype(np.float16)
    W016 = np.asarray(W0, np.float32).astype(np.float16)

    # blobA: [Wqd 4x1024 | qT 4x128 | ident 128] - qT is per-core
    blobA_shared = np.empty((128, 4096 + 4 * QT + 128), np.float16)
    blobA_shared[:, :4096] = chunks(Wqd, 1024)
    blobA_shared[:, 4096 + 4 * QT :] = np.eye(128, dtype=np.float16)
    # blobB: [Wkd 4x1024 | kT 4x256]
    blobB_shared = np.empty((128, 4096 + 4 * L), np.float16)
    blobB_shared[:, :4096] = chunks(Wkd, 1024)
    # blobC: [Wv 4x512 | vT 4x256 | W0 4x512]
    blobC_shared = np.empty((128, 2048 + 4 * L + 2048), np.float16)
    blobC_shared[:, :2048] = chunks(Wv16, 512)
    blobC_shared[:, 2048 + 4 * L :] = chunks(W016, 512)

    in_maps = []
    for c in range(NCORES):
        b, half = c // 2, c % 2
        rows = slice(half * QT, (half + 1) * QT)
        bA = blobA_shared.copy()
        bA[:, 4096 : 4096 + 4 * QT] = chunks(
            np.ascontiguousarray(q[b, rows, :].T).astype(np.float16), QT
        )
        bB = blobB_shared.copy()
        bB[:, 4096:] = chunks(
            np.ascontiguousarray(k[b].T).astype(np.float16), L
        )
        bC = blobC_shared.copy()
        bC[:, 2048 : 2048 + 4 * L] = chunks(
            np.ascontiguousarray(v[b].T).astype(np.float16), L
        )
        m = dict(blob32=blob32, blobA=bA, blobB=bB, blobC=bC)
        if masked:
            m["pen"] = np.ascontiguousarray(
                np.where(mask[b, rows, :] == 0, NEG, 0.0).astype(np.float32)
            )
        in_maps.append(m)
    return in_maps, masked


def kernel(q, k, v, mask, Wq, bq, Wk, bk, Wv, bv, vp, W0, b0):
    in_maps, masked = build_in_maps(
        q, k, v, mask, Wq, bq, Wk, bk, Wv, bv, vp, W0, b0
    )
    if masked not in _compiled:
        _compiled[masked] = _build_nc(masked)
    from concourse.bass_utils import run_bass_kernel_spmd

    res = run_bass_kernel_spmd(
        _compiled[masked], in_maps, core_ids=list(range(NCORES))
    )
    outf = np.zeros((B, L, D), np.float32)
    for c, r in enumerate(res.results):
        b, half = c // 2, c % 2
        outf[b, half * QT : (half + 1) * QT, :] = r["out"]
    return outf


# revision 22
# speedup vs baseline: 1.4741x; 1.1075x over previous
"""Bahdanau multi-head attention on 8 Trainium2 NeuronCores.

Sharding: 8 shards = (batch B=4) x (query-half Lq=128). Each core owns ALL
heads for its 128 query rows, so the W0 output projection is fully local.

Math: tanh(q+k) is expanded in a 6-term sine series at odd harmonics of a
base frequency (tanh(x) ~ sum_m c_m sin((2m-1) w x)), which turns the
Bahdanau score contraction into PE matmuls over (sin/cos feature, dk)
pairs instead of a 16.7M-element ScalarE tanh:

  scores[q,k] = sum_d vp_d tanh(qh_d + kh_d)
             ~= sum_m c_m sum_d vp_d [sin_m(q)cos_m(k) + cos_m(q)sin_m(k)]

Per-core device algorithm (L=256 keys, H=8 heads, DK=64):
  1. Duplicated projections on PE (fp16, FWL): psum qh2[h] = [128=(2 x dk),
     128 q] per head (dk rows duplicated so sin/cos share one tile), ACT
     Copy+per-head-bias moves them to SBUF fp16. Same for kh2 (256 cols).
  2. Seed features via ACT Sin with per-partition phase {0, pi/2}:
     F1 = [sin(w x); cos(w x)] (args <= 3.06 < pi, inside the Sin table's
     valid range). ACT Square + DVE tensor_scalar give C2 = 2cos(2 w x).
  3. Chebyshev ladder on DVE (fp16 2x): F3 = F1*(C2 +- 1),
     F{m+2} = C2*Fm - F{m-2} produces odd harmonics {1,3,5,7,9,11}.
     vp folds into the q-side seed, c_m applied per harmonic.
  4. Scores: 6 matmuls per head accumulate [128 q, 256 k] f32 in PSUM with
     queries on partitions - the natural softmax orientation, no transpose.
  5. Softmax without max-subtraction (|scores| <= ~6 so exp is safe):
     ACT Exp straight from PSUM, DVE rowsum/reciprocal/scale.
  6. attn transposed via plain matmul against identity (faster than
     transpose-mode), attn @ vh on PE, W0 projection, +b0, DMA out.
"""

import numpy as np

B, L, D, H, DK = 4, 256, 512, 8, 64
NCORES = 8
QT = 128          # query rows per core
NCH = D // 128    # 4 chunks of 128 along D
NEG = -30.0       # masked-score penalty (exp(-30) ~ 1e-13)

OM_HALF = 0.300
HARMONICS = (1, 3, 5, 7, 9)
M = len(HARMONICS)


def _fit_coeffs():
    om = np.array([m * OM_HALF for m in HARMONICS])
    x = np.linspace(-9.9, 9.9, 4001)
    wgt = np.exp(-x * x / 4.0) + 3e-3
    A = np.sin(np.outer(x, om))
    w = np.sqrt(wgt)[:, None]
    c, *_ = np.linalg.lstsq(A * w, np.tanh(x) * w[:, 0], rcond=None)
    return c.astype(np.float64)


_C = _fit_coeffs()

_compiled = {}


def _build_nc(masked):
    import concourse.mybir as mybir
    import concourse.tile as tile
    from concourse import bacc

    f32 = mybir.dt.float32
    f16 = mybir.dt.float16
    AF = mybir.ActivationFunctionType
    ALU = mybir.AluOpType
    AX = mybir.AxisListType

    nc = bacc.Bacc(
        "TRN2",
        target_bir_lowering=False,
        debug=False,
        enable_asserts=False,
        num_devices=NCORES,
    )

    # consolidated input blobs - one DMA each (HWDGE charges ~625ns per
    # DMA regardless of size, so 35 small DMAs would serialize ~22us):
    # blob32: [sbq 8 | sbk 8 | vpdup 8 | ppc 6 | bvB 512 | b0B 512]
    # blobA:  [Wqd 4x1024 | qT 4x128 | ident 128]
    # blobB:  [Wkd 4x1024 | kT 4x256]
    # blobC:  [Wv 4x512 | vT 4x256 | W0 4x512]
    N32 = 1054
    NA = 4 * 1024 + 4 * QT + 128
    NB = 4 * 1024 + 4 * L
    NC = 2048 + 4 * L + 2048
    blob32 = nc.dram_tensor("blob32", [128, N32], f32, kind="ExternalInput").ap()
    blobA = nc.dram_tensor("blobA", [128, NA], f16, kind="ExternalInput").ap()
    blobB = nc.dram_tensor("blobB", [128, NB], f16, kind="ExternalInput").ap()
    blobC = nc.dram_tensor("blobC", [128, NC], f16, kind="ExternalInput").ap()
    if masked:
        pen = nc.dram_tensor("pen", [QT, L], f32, kind="ExternalInput").ap()
    out = nc.dram_tensor("out", [QT, D], f32, kind="ExternalOutput").ap()

    NQ = H * QT   # 1024 q-side feature cols
    NK = H * L    # 2048 k-side feature cols

    with tile.TileContext(nc) as tc:
        with tc.tile_pool(name="const", bufs=1) as cp:
            t32 = cp.tile([128, N32], f32, tag="t32")
            nc.sync.dma_start(t32[:], blob32[:])
            tA = cp.tile([128, NA], f16, tag="tA")
            nc.sync.dma_start(tA[:], blobA[:])
            tB = cp.tile([128, NB], f16, tag="tB")
            nc.sync.dma_start(tB[:], blobB[:])
            tC = cp.tile([128, NC], f16, tag="tC")
            nc.sync.dma_start(tC[:], blobC[:])
            if masked:
                pen_t = cp.tile([QT, L], f32, tag="pen")
                nc.sync.dma_start(pen_t[:], pen[:])

            sbq_t = t32[:, 0:H]
            sbk_t = t32[:, H : 2 * H]
            vpdup_t = t32[:, 2 * H : 3 * H]
            ppc_t = t32[:, 3 * H : 3 * H + 6]
            bvB_t = t32[:, 30 : 30 + D]
            b0B_t = t32[:, 30 + D : 30 + 2 * D]
            Wqd_t = [tA[:, c * 1024 : (c + 1) * 1024] for c in range(NCH)]
            qT_t = [tA[:, 4096 + c * QT : 4096 + (c + 1) * QT] for c in range(NCH)]
            ident_t = tA[:, 4096 + 4 * QT : 4096 + 4 * QT + 128]
            Wkd_t = [tB[:, c * 1024 : (c + 1) * 1024] for c in range(NCH)]
            kT_t = [tB[:, 4096 + c * L : 4096 + (c + 1) * L] for c in range(NCH)]
            Wv_t = [tC[:, c * 512 : (c + 1) * 512] for c in range(NCH)]
            vT_t = [tC[:, 2048 + c * L : 2048 + (c + 1) * L] for c in range(NCH)]
            W0_t = [tC[:, 3072 + c * 512 : 3072 + (c + 1) * 512] for c in range(NCH)]

            vh16 = [cp.tile([128, D], f16, tag=f"vh{j}", name=f"vh{j}") for j in range(2)]
            F1q = cp.tile([128, NQ], f16, tag="F1q")
            F1k = cp.tile([128, NK], f16, tag="F1k")

            # ---- duplicated projections fused with seed Sin ----
            with (
                tc.tile_pool(name="psq", bufs=2, space="PSUM") as psqp,
                tc.tile_pool(name="psk", bufs=2, space="PSUM") as pskp,
                tc.tile_pool(name="psv", bufs=2, space="PSUM") as psvp,
            ):
                for h in range(H):
                    hs = slice(h * 128, (h + 1) * 128)
                    psq = psqp.tile([128, QT], f32, tag="psq", name=f"psq{h}")
                    for c in range(NCH):
                        nc.tensor.matmul(
                            psq[:], lhsT=Wqd_t[c][:, hs], rhs=qT_t[c][:],
                            start=(c == 0), stop=(c == NCH - 1),
                        )
                    nc.scalar.activation(
                        F1q[:, hs], psq[:], AF.Sin,
                        bias=sbq_t[:, h : h + 1], scale=OM_HALF,
                    )
                    ks = slice(h * L, (h + 1) * L)
                    psk = pskp.tile([128, L], f32, tag="psk", name=f"psk{h}")
                    for c in range(NCH):
                        nc.tensor.matmul(
                            psk[:], lhsT=Wkd_t[c][:, hs], rhs=kT_t[c][:],
                            start=(c == 0), stop=(c == NCH - 1),
                        )
                    nc.scalar.activation(
                        F1k[:, ks], psk[:], AF.Sin,
                        bias=sbk_t[:, h : h + 1], scale=OM_HALF,
                    )
                for j in range(2):
                    js = slice(j * 128, (j + 1) * 128)
                    psv = psvp.tile([128, D], f32, tag="psv", name=f"psv{j}")
                    for c in range(NCH):
                        nc.tensor.matmul(
                            psv[:], lhsT=vT_t[c][:, js], rhs=Wv_t[c][:],
                            start=(c == 0), stop=(c == NCH - 1),
                        )
                    nc.vector.tensor_add(vh16[j][:], psv[:], bvB_t[:])

            # ---- ladder multipliers: C2 = 2cos(2 w x), M3 = C2 -+ 1 ----
            # computed per column-half (heads 0-3 / 4-7) so the first half's
            # scores+softmax can overlap the second half's ladder
            def mults(F1, n, sfx, o):
                mp = ppc_t[:, o : o + 1]
                ap = ppc_t[:, o + 1 : o + 2]
                pm1 = ppc_t[:, o + 2 : o + 3]
                SQ = cp.tile([128, n], f16, tag=f"SQ{sfx}", name=f"SQ{sfx}")
                nc.scalar.activation(SQ[:], F1[:], AF.Square)
                C2 = cp.tile([128, n], f16, tag=f"C2{sfx}", name=f"C2{sfx}")
                nc.vector.tensor_scalar(C2[:], SQ[:], mp, ap, ALU.mult, ALU.add)
                M3 = cp.tile([128, n], f16, tag=f"M3{sfx}", name=f"M3{sfx}")
                nc.vector.tensor_scalar(M3[:], C2[:], pm1, None, ALU.add)
                return C2, M3

            C2q, M3q = mults(F1q, NQ, "q", 0)
            C2k, M3k = mults(F1k, NK, "k", 3)

            Gq = [cp.tile([128, NQ], f16, tag=f"Gq{i}", name=f"Gq{i}") for i in range(M)]
            Hq = [cp.tile([128, NQ], f16, tag=f"Hq{i}", name=f"Hq{i}") for i in range(M)]
            Kk = [None] * M
            Kk[0] = F1k
            Kk[1] = cp.tile([128, NK], f16, tag="Kk1", name="Kk1")
            for i in range(2, M):
                Kk[i] = cp.tile([128, NK], f16, tag=f"Kk{i}", name=f"Kk{i}")
            tmpq = [cp.tile([128, NQ], f16, tag=f"tq{i}", name=f"tq{i}") for i in range(M - 2)]
            tmpk = [cp.tile([128, NK], f16, tag=f"tk{i}", name=f"tk{i}") for i in range(M - 2)]

            with (
                tc.tile_pool(name="sc", bufs=4, space="PSUM") as scp,
                tc.tile_pool(name="tr", bufs=2, space="PSUM") as trp,
                tc.tile_pool(name="av", bufs=1, space="PSUM") as avp,
                tc.tile_pool(name="smp", bufs=3) as smp,
                tc.tile_pool(name="ops", bufs=1, space="PSUM") as opsp,
            ):
                # two heads per PSUM bank: head h lives in sc_ps[h//2]
                # cols [(h%2)*L, (h%2+1)*L). start=True clears has_written
                # bank-wide, so only the bank's FIRST matmul (h even, i=0)
                # sets it; the odd head's first write lands on cleared bits
                # and overwrites correctly with start=False.
                sc_ps = [
                    scp.tile([128, 2 * L], f32, tag="sc", name=f"sc{hp}")
                    for hp in range(H // 2)
                ]

                def sc_ap(h):
                    return sc_ps[h // 2][:, (h % 2) * L : (h % 2 + 1) * L]

                def ladder_dve(i, half):
                    hq = slice(half * NQ // 2, (half + 1) * NQ // 2)
                    hk = slice(half * NK // 2, (half + 1) * NK // 2)
                    if i == 0:
                        for h in range(4 * half, 4 * half + 4):
                            hs = slice(h * 128, (h + 1) * 128)
                            nc.vector.tensor_scalar_mul(
                                Gq[0][:, hs], F1q[:, hs], vpdup_t[:, h : h + 1]
                            )
                    elif i == 1:
                        nc.vector.tensor_mul(Gq[1][:, hq], Gq[0][:, hq], M3q[:, hq])
                        nc.vector.tensor_mul(Kk[1][:, hk], F1k[:, hk], M3k[:, hk])
                    else:
                        nc.vector.tensor_mul(tmpq[i - 2][:, hq], C2q[:, hq], Gq[i - 1][:, hq])
                        nc.vector.tensor_sub(Gq[i][:, hq], tmpq[i - 2][:, hq], Gq[i - 2][:, hq])
                        nc.vector.tensor_mul(tmpk[i - 2][:, hk], C2k[:, hk], Kk[i - 1][:, hk])
                        nc.vector.tensor_sub(Kk[i][:, hk], tmpk[i - 2][:, hk], Kk[i - 2][:, hk])
                    nc.scalar.activation(Hq[i][:, hq], Gq[i][:, hq], AF.Copy,
                                         scale=float(_C[i]))
                    for h in range(4 * half, 4 * half + 4):
                        nc.tensor.matmul(
                            sc_ap(h),
                            lhsT=Hq[i][:, h * 128 : (h + 1) * 128],
                            rhs=Kk[i][:, h * L : (h + 1) * L],
                            start=(i == 0 and h % 2 == 0),
                            stop=(i == M - 1),
                        )

                aoT = [cp.tile([128, QT], f16, tag=f"aoT{c}", name=f"aoT{c}") for c in range(NCH)]
                ops = opsp.tile([QT, D], f32, tag="ops")

                def softmax_head(h):
                    if masked:
                        spen = smp.tile([QT, L], f32, tag="spen")
                        nc.vector.tensor_add(spen[:], sc_ap(h), pen_t[:])
                        p = smp.tile([QT, L], f16, tag="p")
                        nc.scalar.activation(p[:], spen[:], AF.Exp)
                    else:
                        p = smp.tile([QT, L], f16, tag="p")
                        nc.scalar.activation(p[:], sc_ap(h), AF.Exp)
                    rs = smp.tile([QT, 1], f32, tag="rs")
                    nc.vector.tensor_reduce(rs[:], p[:], axis=AX.X, op=ALU.add)
                    rcp = smp.tile([QT, 1], f32, tag="rcp")
                    nc.vector.reciprocal_approx_fast(rcp[:], rs[:])
                    attn = smp.tile([QT, L], f16, tag="attn")
                    nc.vector.tensor_scalar_mul(attn[:], p[:], rcp[:])
                    av = avp.tile([64, QT], f32, tag="av", name=f"av{h}")
                    for j in range(2):
                        js = slice(j * 128, (j + 1) * 128)
                        tr = trp.tile([128, QT], f32, tag="tr")
                        nc.tensor.matmul(
                            tr[:], lhsT=attn[:, js], rhs=ident_t,
                            start=True, stop=True,
                        )
                        attnT = smp.tile([128, QT], f16, tag="attnT")
                        nc.scalar.activation(attnT[:], tr[:], AF.Copy)
                        nc.tensor.matmul(
                            av[:], lhsT=vh16[j][:, h * 64 : (h + 1) * 64],
                            rhs=attnT[:], start=(j == 0), stop=(j == 1),
                        )
                    hh = h % 2
                    nc.vector.tensor_copy(
                        aoT[h // 2][hh * 64 : (hh + 1) * 64, :], av[:]
                    )
                    if hh == 1:
                        c = h // 2
                        nc.tensor.matmul(
                            ops[:], lhsT=aoT[c][:], rhs=W0_t[c],
                            start=(c == 0), stop=(c == NCH - 1),
                        )

                # half 0 ladder+scores, then half 1 interleaved with half 0's
                # softmax so ACT/PE softmax work hides under the DVE ladder
                for i in range(M):
                    ladder_dve(i, 0)
                for i in range(M):
                    ladder_dve(i, 1)
                    if i >= 1:
                        softmax_head(i - 1)
                for h in range(4, 8):
                    softmax_head(h)

                y = smp.tile([QT, D], f32, tag="y")
                nc.vector.tensor_add(y[:], ops[:], b0B_t[:])
                nc.sync.dma_start(out[:], y[:])

    nc.compile()
    return nc


def _dup_cols(W):
    # [D, D] -> [D, H*128] with cols h*128 + r*64 + j = W[:, h*64 + j]
    idx = np.arange(H * 128)
    src = (idx // 128) * 64 + (idx % 64)
    return np.ascontiguousarray(W[:, src])


def _dup_part(v):
    # [D] -> [128, H] with [p, h] = v[h*64 + p%64]
    p = np.arange(128) % 64
    return np.ascontiguousarray(v[p[:, None] + np.arange(H)[None, :] * 64])


def build_in_maps(q, k, v, mask, Wq, bq, Wk, bk, Wv, bv, vp, W0, b0):
    q = np.asarray(q, np.float32)
    k = np.asarray(k, np.float32)
    v = np.asarray(v, np.float32)
    mask = np.asarray(mask)
    vp = np.asarray(vp, np.float32).reshape(H, DK)

    masked = not bool(np.all(mask != 0))

    # blob32: [sbq 8 | sbk 8 | vpdup 8 | ppc 6 | bvB 512 | b0B 512]
    blob32 = np.zeros((128, 1054), np.float32)
    phq = np.zeros((128, 1), np.float32)
    phq[64:] = np.pi / 2                 # q: [sin; cos]
    phk = np.zeros((128, 1), np.float32)
    phk[:64] = np.pi / 2                 # k: [cos; sin]
    blob32[:, 0:H] = OM_HALF * _dup_part(np.asarray(bq, np.float32)) + phq
    blob32[:, H : 2 * H] = OM_HALF * _dup_part(np.asarray(bk, np.float32)) + phk
    blob32[:, 2 * H : 3 * H] = vp.T[np.arange(128) % 64]
    ppc = blob32[:, 3 * H : 3 * H + 6]
    # q side ([sin; cos]): mp {-4, 4}, ap {2, -2}, pm1 {1, -1}
    ppc[:64, 0], ppc[64:, 0] = -4.0, 4.0
    ppc[:64, 1], ppc[64:, 1] = 2.0, -2.0
    ppc[:64, 2], ppc[64:, 2] = 1.0, -1.0
    # k side ([cos; sin]): halves swapped
    ppc[:64, 3], ppc[64:, 3] = 4.0, -4.0
    ppc[:64, 4], ppc[64:, 4] = -2.0, 2.0
    ppc[:64, 5], ppc[64:, 5] = -1.0, 1.0
    blob32[:, 30:542] = np.asarray(bv, np.float32)[None, :]
    blob32[:, 542:1054] = np.asarray(b0, np.float32)[None, :]

    def chunks(W, n):
        # [512, n] -> [128, 4*n] chunk-major
        return W.reshape(NCH, 128, n).transpose(1, 0, 2).reshape(128, NCH * n)

    Wqd = _dup_cols(np.asarray(Wq, np.float32)).astype(np.float16)
    Wkd = _dup_cols(np.asarray(Wk, np.float32)).astype(np.float16)
    Wv16 = np.asarray(Wv, np.float32).astype(np.float16)
    W016 = np.asarray(W0, np.float32).astype(np.float16)

    # blobA: [Wqd 4x1024 | qT 4x128 | ident 128] - qT is per-core
    blobA_shared = np.empty((128, 4096 + 4 * QT + 128), np.float16)
    blobA_shared[:, :4096] = chunks(Wqd, 1024)
    blobA_shared[:, 4096 + 4 * QT :] = np.eye(128, dtype=np.float16)
    # blobB: [Wkd 4x1024 | kT 4x256]
    blobB_shared = np.empty((128, 4096 + 4 * L), np.float16)
    blobB_shared[:, :4096] = chunks(Wkd, 1024)
    # blobC: [Wv 4x512 | vT 4x256 | W0 4x512]
    blobC_shared = np.empty((128, 2048 + 4 * L + 2048), np.float16)
    blobC_shared[:, :2048] = chunks(Wv16, 512)
    blobC_shared[:, 2048 + 4 * L :] = chunks(W016, 512)

    in_maps = []
    for c in range(NCORES):
        b, half = c // 2, c % 2
        rows = slice(half * QT, (half + 1) * QT)
        bA = blobA_shared.copy()
        bA[:, 4096 : 4096 + 4 * QT] = chunks(
            np.ascontiguousarray(q[b, rows, :].T).astype(np.float16), QT
        )
        bB = blobB_shared.copy()
        bB[:, 4096:] = chunks(
            np.ascontiguousarray(k[b].T).astype(np.float16), L
        )
        bC = blobC_shared.copy()
        bC[:, 2048 : 2048 + 4 * L] = chunks(
            np.ascontiguousarray(v[b].T).astype(np.float16), L
        )
        m = dict(blob32=blob32, blobA=bA, blobB=bB, blobC=bC)
        if masked:
            m["pen"] = np.ascontiguousarray(
                np.where(mask[b, rows, :] == 0, NEG, 0.0).astype(np.float32)
            )
        in_maps.append(m)
    return in_maps, masked


def kernel(q, k, v, mask, Wq, bq, Wk, bk, Wv, bv, vp, W0, b0):
    in_maps, masked = build_in_maps(
        q, k, v, mask, Wq, bq, Wk, bk, Wv, bv, vp, W0, b0
    )
    if masked not in _compiled:
        _compiled[masked] = _build_nc(masked)
    from concourse.bass_utils import run_bass_kernel_spmd

    res = run_bass_kernel_spmd(
        _compiled[masked], in_maps, core_ids=list(range(NCORES))
    )
    outf = np.zeros((B, L, D), np.float32)
    for c, r in enumerate(res.results):
        b, half = c // 2, c % 2
        outf[b, half * QT : (half + 1) * QT, :] = r["out"]
    return outf
